# revision 13
# baseline (speedup 1.0000x reference)
"""Trainium2 Bass kernel for CollaborativeWaterfallMoE.

Strategy: data-parallel over batch (64 images per core on 8 cores).
The device computes the shared conv encoder + dense per-expert conv heads
(channels on partitions, pixels on the free dim, 3x3 convs as 9 PSUM-
accumulated shifted matmuls; conv1 via im2col with K=27).  The device
returns, per core, the pooled feature vector sums ("flat") and the
per-expert pooled embedding sums ("zf").  The tiny [B,E] scorer +
waterfall routing + per-expert FC heads + one-hot combine run on host.
"""

import os
import sys
import time

import numpy as np

sys.path.insert(0, "/opt/trn_rl_repo")

import concourse.bass as bass
import concourse.mybir as mybir
import concourse.tile as tile
from concourse.vector_clock import ScopedClock
from concourse.bass_utils import run_bass_kernel_spmd

# ----------------------------------------------------------------------------
# Patch: the walrus build in this container rejects >1 sync wait on a CTRL
# (Drain) instruction; split the TileContext tail-drain waits across a chain
# of drain instructions (one wait each).
# ----------------------------------------------------------------------------
_MAX_WAITS = 1


def _patched_drain_and_barrier(self, tick_clock, wait_clock):
    drain_inst = self.nc.sync.drain()
    wait_clock.add_sem_waits(
        drain_inst.ins, ScopedClock({None: tick_clock.global_clock})
    )
    si = drain_inst.ins.sync_info
    waits = list(si.on_wait or [])
    if len(waits) > _MAX_WAITS:
        si.on_wait = waits[:_MAX_WAITS]
        rest = waits[_MAX_WAITS:]
        while rest:
            d2 = self.nc.sync.drain()
            si2 = d2.ins.sync_info
            if si2 is None:
                d2.ins.sync_info = mybir.SyncInfo(
                    on_wait=rest[:_MAX_WAITS], on_update=[]
                )
            else:
                si2.on_wait = rest[:_MAX_WAITS]
            rest = rest[_MAX_WAITS:]
    self.nc.all_engine_barrier()
    assert self.sems is not None
    popped = self.nc._tile_sem_poison_stack.pop()
    assert popped is self._sem_poison
    self.nc.clear_and_free_semaphores(list(self.sems.allocated().values()))
    self.nc.all_engine_barrier()


tile.TileContext._drain_and_barrier = _patched_drain_and_barrier

# ----------------------------------------------------------------------------
# Problem constants (hardcoded; must match the grader's reference.py)
# ----------------------------------------------------------------------------
E = 4
C1, C2, C3 = 64, 128, 256
EMB, NC_CLS, SH = 256, 10, 128
B, H, W = 512, 64, 64
CAP = 128
T_ROUTE = 0.1
EPS = 1e-5
N_CORES = 8
N_IMG = B // N_CORES  # 64 images per core total
# Images per core per launch.  The program is fully unrolled, so NEFF size
# (and neuronxcc compile time) scales with this; the full batch is covered
# by N_IMG // N_IMG_LAUNCH sequential launches of the same compiled program.
N_IMG_LAUNCH = 8

F32 = mybir.dt.float32
F32R = mybir.dt.float32r

# Encoder convs run in exact fp32 (the waterfall routing argmax gaps are
# ~1e-5, so "flat" must match the reference to fp32 accuracy).  The expert
# conv heads only feed the output logits and run in float32r (~1e-4 rel).
USE_F32R_EXPERTS = True

OFFS = [(k // 3, k % 3) for k in range(9)]  # (dy, dx)


def split_multi_waits(nc, max_waits=1):
    """walrus in this container rejects >1 sync wait per instruction; move
    excess waits onto single-wait NOPs inserted just before the instruction."""
    eng_map = {
        mybir.EngineType.PE: nc.tensor,
        mybir.EngineType.Activation: nc.scalar,
        mybir.EngineType.DVE: nc.vector,
        mybir.EngineType.Pool: nc.gpsimd,
        mybir.EngineType.SP: nc.sync,
    }
    f = nc.m.functions[0]
    for bb in f.blocks:
        insts = list(bb.instructions)
        out = []
        changed = False
        for inst in insts:
            si = getattr(inst, "sync_info", None)
            waits = list(si.on_wait) if si is not None and si.on_wait else []
            if len(waits) > max_waits and inst.engine in eng_map:
                for w in waits[max_waits:]:
                    ni = eng_map[inst.engine].nop().ins
                    cur = nc.cur_bb.bb
                    last = cur.instructions[-1]
                    assert last.name == ni.name
                    cur.instructions.pop()
                    if ni.sync_info is None:
                        ni.sync_info = mybir.SyncInfo(on_wait=[w], on_update=[])
                    else:
                        ni.sync_info.on_wait = [w]
                    out.append(ni)
                si.on_wait = waits[:max_waits]
                changed = True
            out.append(inst)
        if changed:
            bb.instructions[:] = out


# ----------------------------------------------------------------------------
# Device program
# ----------------------------------------------------------------------------
def build_nc(n_img=N_IMG):
    nc = bass.Bass("TRN2", target_bir_lowering=False, debug=False)

    x_d = nc.declare_dram_parameter("x", [n_img, 3, H, W], F32, isOutput=False)
    w1_d = nc.declare_dram_parameter("w1", [27, C1], F32, isOutput=False)
    w2_d = nc.declare_dram_parameter("w2", [C1, 9 * C1], F32, isOutput=False)
    w3_d = nc.declare_dram_parameter("w3", [C1, 9 * C2], F32, isOutput=False)
    w4_d = nc.declare_dram_parameter("w4", [C2, 9 * C2], F32, isOutput=False)
    cw_d = nc.declare_dram_parameter("cw", [C2, 8 * 9 * 128], F32, isOutput=False)
    scl_d = nc.declare_dram_parameter("scl", [128, 12], F32, isOutput=False)
    # scl columns: 0=s1,1=t1 (rows 0:64), 2=s2,3=t2 (rows 0:64),
    #              4=s3,5=t3, 6=s4,7=t4, 8..11 unused
    scle_d = nc.declare_dram_parameter("scle", [128, 8], F32, isOutput=False)
    biae_d = nc.declare_dram_parameter("biae", [128, 8], F32, isOutput=False)

    flat_o = nc.declare_dram_parameter("flat_out", [C2, n_img], F32, isOutput=True)
    zf_o = nc.declare_dram_parameter("zf_out", [128, E * 2 * n_img], F32, isOutput=True)

    with tile.TileContext(nc) as tc:
        with (
            tc.tile_pool(name="persist", bufs=1) as pp,
            tc.tile_pool(name="rows", bufs=3) as rowp,
            tc.tile_pool(name="tmp", bufs=3) as tmpp,
            tc.tile_pool(name="ebuf", bufs=2) as ebp,
            tc.tile_pool(name="psum", bufs=6, space="PSUM") as psp,
        ):
            # ---- persistent weight / scale tiles ----
            w1s = pp.tile([27, C1], F32, tag="w1s", name="w1s")
            w2s = pp.tile([C1, 9 * C1], F32, tag="w2s", name="w2s")
            w3s = pp.tile([C1, 9 * C2], F32, tag="w3s", name="w3s")
            w4s = pp.tile([C2, 9 * C2], F32, tag="w4s", name="w4s")
            cws = pp.tile([C2, 8 * 9 * 128], F32, tag="cws", name="cws")
            scls = pp.tile([128, 12], F32, tag="scls", name="scls")
            scles = pp.tile([128, 8], F32, tag="scles", name="scles")
            biaes = pp.tile([128, 8], F32, tag="biaes", name="biaes")
            nc.sync.dma_start(w1s[:], w1_d[:])
            nc.sync.dma_start(w2s[:], w2_d[:])
            nc.sync.dma_start(w3s[:], w3_d[:])
            nc.sync.dma_start(w4s[:], w4_d[:])
            nc.sync.dma_start(cws[:], cw_d[:])
            nc.sync.dma_start(scls[:], scl_d[:])
            nc.sync.dma_start(scles[:], scle_d[:])
            nc.sync.dma_start(biaes[:], biae_d[:])

            s1 = scls[0:C1, 0:1]
            t1 = scls[0:C1, 1:2]
            s2 = scls[0:C1, 2:3]
            t2 = scls[0:C1, 3:4]
            s3 = scls[0:C2, 4:5]
            t3 = scls[0:C2, 5:6]
            s4 = scls[0:C2, 6:7]
            t4 = scls[0:C2, 7:8]

            # ---- persistent activations (double buffered, zero borders) ----
            t27 = [pp.tile([27, H, W], F32, tag=f"t27_{q}", name=f"t27_{q}") for q in range(2)]
            c1o = [pp.tile([C1, H + 2, W + 2], F32, tag=f"c1o_{q}", name=f"c1o_{q}") for q in range(2)]
            c3i = [pp.tile([C1, 34, 34], F32, tag=f"c3i_{q}", name=f"c3i_{q}") for q in range(2)]
            c4i = [pp.tile([C2, 34, 34], F32, tag=f"c4i_{q}", name=f"c4i_{q}") for q in range(2)]
            fpr = [pp.tile([C2, 2, 18, 18], F32, tag=f"fpr_{q}", name=f"fpr_{q}") for q in range(2)]
            if USE_F32R_EXPERTS:
                fprr = [
                    pp.tile([C2, 2, 18, 18], F32R, tag=f"fprr_{q}", name=f"fprr_{q}")
                    for q in range(2)
                ]
                cwsr = pp.tile([C2, 8 * 9 * 128], F32R, tag="cwsr", name="cwsr")
                nc.vector.tensor_copy(cwsr[:], cws[:])
            else:
                fprr = fpr
                cwsr = cws
            for q in range(2):
                nc.gpsimd.memset(t27[q][:], 0.0)
                nc.gpsimd.memset(c1o[q][:], 0.0)
                nc.gpsimd.memset(c3i[q][:], 0.0)
                nc.gpsimd.memset(c4i[q][:], 0.0)
                nc.gpsimd.memset(fpr[q][:], 0.0)
                if USE_F32R_EXPERTS:
                    nc.gpsimd.memset(fprr[q][:].bitcast(F32), 0.0)

            flatb = pp.tile([C2, n_img], F32, tag="flatb", name="flatb")
            zfb = pp.tile([128, E, 2, n_img], F32, tag="zfb", name="zfb")

            for i in range(n_img):
                q = i % 2
                tt = t27[q]
                co = c1o[q]
                p3 = c3i[q]
                p4 = c4i[q]
                fp = fpr[i // 2 % 2]

                # ---- im2col for conv1: 9 shifted copies of the 3-ch image ----
                for k, (dy, dx) in enumerate(OFFS):
                    ly, hy = max(0, 1 - dy), min(H, H + 1 - dy)
                    lx, hx = max(0, 1 - dx), min(W, W + 1 - dx)
                    nc.sync.dma_start(
                        tt[3 * k : 3 * k + 3, ly:hy, lx:hx],
                        x_d[i, :, ly + dy - 1 : hy + dy - 1, lx + dx - 1 : hx + dx - 1],
                    )

                # ---- conv1: K=27 im2col matmul, 8 N-tiles of 512 ----
                for n in range(8):
                    ps = psp.tile([128, 512], F32, tag="mm", name="mm")
                    nc.tensor.matmul(
                        ps[0:C1, :],
                        w1s[:],
                        tt[:, 8 * n : 8 * n + 8, :],
                        start=True,
                        stop=True,
                    )
                    nc.scalar.activation(
                        co[:, 1 + 8 * n : 9 + 8 * n, 1 : 1 + W],
                        ps[0:C1, :].rearrange("p (a b) -> p a b", a=8),
                        mybir.ActivationFunctionType.Relu,
                        bias=t1,
                        scale=s1,
                    )

                # ---- conv2 (64->64) + maxpool -> c3i ----
                for n in range(8):
                    ps = psp.tile([128, 512], F32, tag="mm", name="mm")
                    for k, (dy, dx) in enumerate(OFFS):
                        nc.tensor.matmul(
                            ps[0:C1, :],
                            w2s[:, k * C1 : (k + 1) * C1],
                            co[:, 8 * n + dy : 8 * n + dy + 8, dx : dx + W],
                            start=(k == 0),
                            stop=(k == 8),
                        )
                    c2r = rowp.tile([C1, 8, W], F32, tag="c2r", name="c2r")
                    nc.scalar.activation(
                        c2r[:],
                        ps[0:C1, :].rearrange("p (a b) -> p a b", a=8),
                        mybir.ActivationFunctionType.Relu,
                        bias=t2,
                        scale=s2,
                    )
                    tm = tmpp.tile([C1, 8, 32], F32, tag="tm1", name="tm1")
                    c2rs = c2r[:].rearrange("p a (b two) -> p a b two", two=2)
                    nc.vector.tensor_max(tm[:], c2rs[:, :, :, 0], c2rs[:, :, :, 1])
                    tms = tm[:].rearrange("p (a two) b -> p a two b", two=2)
                    nc.vector.tensor_max(
                        p3[:, 1 + 4 * n : 5 + 4 * n, 1:33],
                        tms[:, :, 0, :],
                        tms[:, :, 1, :],
                    )

                # ---- conv3 (64->128) ----
                for n in range(2):
                    ps = psp.tile([128, 512], F32, tag="mm", name="mm")
                    for k, (dy, dx) in enumerate(OFFS):
                        nc.tensor.matmul(
                            ps[:],
                            w3s[:, k * C2 : (k + 1) * C2],
                            p3[:, 16 * n + dy : 16 * n + dy + 16, dx : dx + 32],
                            start=(k == 0),
                            stop=(k == 8),
                        )
                    nc.scalar.activation(
                        p4[:, 1 + 16 * n : 17 + 16 * n, 1:33],
                        ps[:].rearrange("p (a b) -> p a b", a=16),
                        mybir.ActivationFunctionType.Relu,
                        bias=t3,
                        scale=s3,
                    )

                # ---- conv4 (128->128) + maxpool -> fpr ----
                for n in range(2):
                    ps = psp.tile([128, 512], F32, tag="mm", name="mm")
                    for k, (dy, dx) in enumerate(OFFS):
                        nc.tensor.matmul(
                            ps[:],
                            w4s[:, k * C2 : (k + 1) * C2],
                            p4[:, 16 * n + dy : 16 * n + dy + 16, dx : dx + 32],
                            start=(k == 0),
                            stop=(k == 8),
                        )
                    c4r = rowp.tile([C2, 16, 32], F32, tag="c4r", name="c4r")
                    nc.scalar.activation(
                        c4r[:],
                        ps[:].rearrange("p (a b) -> p a b", a=16),
                        mybir.ActivationFunctionType.Relu,
                        bias=t4,
                        scale=s4,
                    )
                    tm = tmpp.tile([C2, 16, 16], F32, tag="tm2", name="tm2")
                    c4rs = c4r[:].rearrange("p a (b two) -> p a b two", two=2)
                    nc.vector.tensor_max(tm[:], c4rs[:, :, :, 0], c4rs[:, :, :, 1])
                    tms = tm[:].rearrange("p (a two) b -> p a two b", two=2)
                    nc.vector.tensor_max(
                        fp[:, q, 1 + 8 * n : 9 + 8 * n, 1:17],
                        tms[:, :, 0, :],
                        tms[:, :, 1, :],
                    )

                if q == 1:
                    # ---- per-pair: flat sums + dense expert conv heads ----
                    j = i // 2
                    nc.vector.reduce_sum(
                        flatb[:, 2 * j : 2 * j + 2],
                        fp[:, :, 1:17, 1:17],
                        axis=mybir.AxisListType.XY,
                    )
                    if USE_F32R_EXPERTS:
                        fpx = fprr[i // 2 % 2]
                        nc.vector.tensor_copy(
                            fpx[:, :, 1:17, 1:17], fp[:, :, 1:17, 1:17]
                        )
                    else:
                        fpx = fp
                    for e in range(E):
                        for m in range(2):
                            g = (e * 2 + m) * 9
                            ps = psp.tile([128, 512], F32, tag="mm", name="mm")
                            for k, (dy, dx) in enumerate(OFFS):
                                nc.tensor.matmul(
                                    ps[:],
                                    cwsr[:, (g + k) * 128 : (g + k + 1) * 128],
                                    fpx[:, :, dy : dy + 16, dx : dx + 16],
                                    start=(k == 0),
                                    stop=(k == 8),
                                )
                            eb = ebp.tile([128, 2, 256], F32, tag="eb", name="eb")
                            em = e * 2 + m
                            nc.scalar.activation(
                                eb[:],
                                ps[:].rearrange("p (a b) -> p a b", a=2),
                                mybir.ActivationFunctionType.Relu,
                                bias=biaes[:, em : em + 1],
                                scale=scles[:, em : em + 1],
                            )
                            nc.vector.reduce_sum(
                                zfb[:, e, m, 2 * j : 2 * j + 2],
                                eb[:],
                                axis=mybir.AxisListType.X,
                            )

            nc.sync.dma_start(flat_o[:], flatb[:])
            nc.sync.dma_start(zf_o[:], zfb[:].rearrange("p a b c -> p (a b c)"))

    split_multi_waits(nc)
    return nc


# ----------------------------------------------------------------------------
# Host-side parameter preparation
# ----------------------------------------------------------------------------
def host_prep(params):
    p = {k: np.asarray(v, dtype=np.float32) for k, v in params.items()}

    def fold(g, be, b):
        s = (g / np.sqrt(np.float32(1.0) + np.float32(EPS))).astype(np.float32)
        t = (b * s + be).astype(np.float32)
        return s, t

    s1, t1 = fold(p["g1"], p["be1"], p["b1"])
    s2, t2 = fold(p["g2"], p["be2"], p["b2"])
    s3, t3 = fold(p["g3"], p["be3"], p["b3"])
    s4, t4 = fold(p["g4"], p["be4"], p["b4"])

    scl = np.zeros((128, 12), np.float32)
    scl[:C1, 0] = s1
    scl[:C1, 1] = t1
    scl[:C1, 2] = s2
    scl[:C1, 3] = t2
    scl[:C2, 4] = s3
    scl[:C2, 5] = t3
    scl[:C2, 6] = s4
    scl[:C2, 7] = t4

    # expert conv scale/bias: columns e*2+m over the 128 channels of m-tile
    se = (p["cg"] / np.sqrt(np.float32(1.0) + np.float32(EPS))).astype(np.float32)
    te = (p["cb"] * se + p["cbe"]).astype(np.float32)
    scle = np.zeros((128, 8), np.float32)
    biae = np.zeros((128, 8), np.float32)
    for e in range(E):
        for m in range(2):
            scle[:, e * 2 + m] = se[e, m * 128 : (m + 1) * 128]
            biae[:, e * 2 + m] = te[e, m * 128 : (m + 1) * 128]

    # t27 partition layout is (k, c): row k*3+c
    w1 = np.ascontiguousarray(p["w1"].transpose(2, 3, 1, 0).reshape(27, C1))
    w2 = np.ascontiguousarray(p["w2"].transpose(1, 2, 3, 0).reshape(C1, 9 * C1))
    w3 = np.ascontiguousarray(p["w3"].transpose(1, 2, 3, 0).reshape(C1, 9 * C2))
    w4 = np.ascontiguousarray(p["w4"].transpose(1, 2, 3, 0).reshape(C2, 9 * C2))
    # cw [E, 256, 128, 3, 3] -> [i, (e, m, k, o)]
    cw = p["cw"].reshape(E, 2, 128, C2, 3, 3)  # [e, m, o, i, dy, dx]
    cw = cw.transpose(3, 0, 1, 4, 5, 2)  # [i, e, m, dy, dx, o]
    cw = np.ascontiguousarray(cw.reshape(C2, 8 * 9 * 128))

    return {
        "w1": w1,
        "w2": w2,
        "w3": w3,
        "w4": w4,
        "cw": cw,
        "scl": scl,
        "scle": scle,
        "biae": biae,
    }, p


# ----------------------------------------------------------------------------
# Host-side tail: scorer, waterfall routing, expert FC heads, combine
# ----------------------------------------------------------------------------
def waterfall(scores_noisy):
    Bn, En = scores_noisy.shape
    NEG = np.float32(-1e9)
    assigned = np.zeros(Bn, bool)
    assignment = np.zeros((Bn, En), bool)
    cap = np.zeros(En, np.int64)
    for it in range(15):
        full = cap >= CAP
        masked = np.where(full[None, :], NEG - np.float32(1.0), scores_noisy)
        masked = np.where(full[None, :], -np.inf, masked)
        best = np.argmax(masked, axis=1)
        quota = 2**it
        for e in range(En):
            want = (~assigned) & (best == e)
            space = min(CAP - cap[e], quota)
            rank = np.cumsum(want.astype(np.int64)) - 1
            sel = want & (rank < space)
            assignment[:, e] |= sel
            cap[e] += sel.sum()
            assigned |= sel
    least = int(np.argmin(cap))
    assignment[:, least] |= ~assigned
    return assignment


def host_tail(flat_sums, zf_sums, p):
    """flat_sums: [B, C2] pixel sums; zf_sums: [E, B, C3] pixel sums."""
    flat = (flat_sums / np.float32(256.0)).astype(np.float32)
    zf = (zf_sums / np.float32(256.0)).astype(np.float32)

    hs = np.einsum("bc,ehc->beh", flat, p["spw"], dtype=np.float32) + p["spb"][None]
    scores = (
        np.einsum("beh,eh->be", hs, p["shw"][:, 0, :], dtype=np.float32)
        + p["shb"][None, :, 0]
    ).astype(np.float32)
    scores = scores - scores.mean(axis=0, keepdims=True)
    combined = scores + np.log(np.float32(1e-9))
    scores_noisy = np.clip(combined / np.float32(T_ROUTE), -1e9, 1e9).astype(
        np.float32
    )

    assignment = waterfall(scores_noisy)

    out = np.zeros((B, NC_CLS), np.float32)
    for e in range(E):
        toks = np.nonzero(assignment[:, e])[0]
        if toks.size == 0:
            continue
        z = np.maximum(zf[e, toks] @ p["pw"][e].T + p["pb"][e], 0.0).astype(np.float32)
        out[toks] = (z @ p["clw"][e].T + p["clb"][e]).astype(np.float32)
    return out


# ----------------------------------------------------------------------------
# Entry point
# ----------------------------------------------------------------------------
_CACHE = {}


def _get_nc():
    if "nc" not in _CACHE:
        _CACHE["nc"] = build_nc(N_IMG_LAUNCH)
    return _CACHE["nc"]


def run_device(x, prep):
    """Run the full batch through the device in several SPMD launches.

    Launch L covers images [L*8*nl, (L+1)*8*nl): core c takes the c-th
    nl-image slice of that window.  Returns (flat_sums [B,C2],
    zf_sums [E,B,C3], per-launch wall times).
    """
    nc = _get_nc()
    nl = N_IMG_LAUNCH
    x = np.ascontiguousarray(np.asarray(x, dtype=np.float32))
    flat_sums = np.zeros((B, C2), np.float32)
    zf_sums = np.zeros((E, B, C3), np.float32)
    n_launch = B // (N_CORES * nl)
    times = []
    for L in range(n_launch):
        base = L * N_CORES * nl
        in_maps = []
        for c in range(N_CORES):
            m = dict(prep)
            m["x"] = np.ascontiguousarray(x[base + c * nl : base + (c + 1) * nl])
            in_maps.append(m)
        t0 = time.perf_counter()
        results = run_bass_kernel_spmd(nc, in_maps, list(range(N_CORES))).results
        times.append(time.perf_counter() - t0)
        for c in range(N_CORES):
            lo = base + c * nl
            fo = results[c]["flat_out"]  # [C2, nl]
            zo = results[c]["zf_out"].reshape(128, E, 2, nl)
            flat_sums[lo : lo + nl] = fo.T
            for e in range(E):
                for m2 in range(2):
                    zf_sums[e, lo : lo + nl, m2 * 128 : (m2 + 1) * 128] = zo[
                        :, e, m2, :
                    ].T
    return flat_sums, zf_sums, times


def kernel(x, params):
    prep, p = host_prep(params)
    flat_sums, zf_sums, _ = run_device(x, prep)
    return host_tail(flat_sums, zf_sums, p)


# revision 21
# speedup vs baseline: 4.9889x; 4.9889x over previous
"""Trainium2 Bass kernel for CollaborativeWaterfallMoE.

Strategy: data-parallel over batch (64 images per core on 8 cores).
The device computes the shared conv encoder + dense per-expert conv heads
(channels on partitions, pixels on the free dim, 3x3 convs as 9 PSUM-
accumulated shifted matmuls; conv1 via im2col with K=27).  The device
returns, per core, the pooled feature vector sums ("flat") and the
per-expert pooled embedding sums ("zf").  The tiny [B,E] scorer +
waterfall routing + per-expert FC heads + one-hot combine run on host.
"""

import os
import sys
import time

import numpy as np

sys.path.insert(0, "/opt/trn_rl_repo")

import concourse.bass as bass
import concourse.mybir as mybir
import concourse.tile as tile
from concourse.vector_clock import ScopedClock
from concourse.bass_utils import run_bass_kernel_spmd

# ----------------------------------------------------------------------------
# Patch: the walrus build in this container rejects >1 sync wait on a CTRL
# (Drain) instruction; split the TileContext tail-drain waits across a chain
# of drain instructions (one wait each).
# ----------------------------------------------------------------------------
_MAX_WAITS = 1


def _patched_drain_and_barrier(self, tick_clock, wait_clock):
    drain_inst = self.nc.sync.drain()
    wait_clock.add_sem_waits(
        drain_inst.ins, ScopedClock({None: tick_clock.global_clock})
    )
    si = drain_inst.ins.sync_info
    waits = list(si.on_wait or [])
    if len(waits) > _MAX_WAITS:
        si.on_wait = waits[:_MAX_WAITS]
        rest = waits[_MAX_WAITS:]
        while rest:
            d2 = self.nc.sync.drain()
            si2 = d2.ins.sync_info
            if si2 is None:
                d2.ins.sync_info = mybir.SyncInfo(
                    on_wait=rest[:_MAX_WAITS], on_update=[]
                )
            else:
                si2.on_wait = rest[:_MAX_WAITS]
            rest = rest[_MAX_WAITS:]
    self.nc.all_engine_barrier()
    assert self.sems is not None
    popped = self.nc._tile_sem_poison_stack.pop()
    assert popped is self._sem_poison
    self.nc.clear_and_free_semaphores(list(self.sems.allocated().values()))
    self.nc.all_engine_barrier()


tile.TileContext._drain_and_barrier = _patched_drain_and_barrier

# ----------------------------------------------------------------------------
# Problem constants (hardcoded; must match the grader's reference.py)
# ----------------------------------------------------------------------------
E = 4
C1, C2, C3 = 64, 128, 256
EMB, NC_CLS, SH = 256, 10, 128
B, H, W = 512, 64, 64
CAP = 128
T_ROUTE = 0.1
EPS = 1e-5
N_CORES = 8
N_IMG = B // N_CORES  # 64 images per core total
# Images per core per launch.  The program is fully unrolled, so NEFF size
# (and neuronxcc compile time) scales with this; the full batch is covered
# by N_IMG // N_IMG_LAUNCH sequential launches of the same compiled program.
N_IMG_LAUNCH = 64

F32 = mybir.dt.float32
F32R = mybir.dt.float32r

# Encoder convs run in exact fp32 (the waterfall routing argmax gaps are
# ~1e-5, so "flat" must match the reference to fp32 accuracy).  The expert
# conv heads only feed the output logits; they run in bf16 (~1e-3 rel),
# which also halves the dominant host->device weight transfer (cw).
EXPERT_DT = "bf16"  # "bf16" | "f32r" | "f32"
BF16 = mybir.dt.bfloat16

OFFS = [(k // 3, k % 3) for k in range(9)]  # (dy, dx)


def split_multi_waits(nc, max_waits=1):
    """walrus in this container rejects >1 sync wait per instruction; move
    excess waits onto single-wait NOPs inserted just before the instruction."""
    eng_map = {
        mybir.EngineType.PE: nc.tensor,
        mybir.EngineType.Activation: nc.scalar,
        mybir.EngineType.DVE: nc.vector,
        mybir.EngineType.Pool: nc.gpsimd,
        mybir.EngineType.SP: nc.sync,
    }
    f = nc.m.functions[0]
    for bb in f.blocks:
        insts = list(bb.instructions)
        out = []
        changed = False
        for inst in insts:
            si = getattr(inst, "sync_info", None)
            waits = list(si.on_wait) if si is not None and si.on_wait else []
            if len(waits) > max_waits and inst.engine in eng_map:
                for w in waits[max_waits:]:
                    ni = eng_map[inst.engine].nop().ins
                    cur = nc.cur_bb.bb
                    last = cur.instructions[-1]
                    assert last.name == ni.name
                    cur.instructions.pop()
                    if ni.sync_info is None:
                        ni.sync_info = mybir.SyncInfo(on_wait=[w], on_update=[])
                    else:
                        ni.sync_info.on_wait = [w]
                    out.append(ni)
                si.on_wait = waits[:max_waits]
                changed = True
            out.append(inst)
        if changed:
            bb.instructions[:] = out


# ----------------------------------------------------------------------------
# Device program
# ----------------------------------------------------------------------------
def build_nc(n_img=N_IMG):
    nc = bass.Bass("TRN2", target_bir_lowering=False, debug=False)

    x_d = nc.declare_dram_parameter("x", [n_img, 3, H, W], F32, isOutput=False)
    w1_d = nc.declare_dram_parameter("w1", [27, 2 * C1], F32, isOutput=False)
    w2_d = nc.declare_dram_parameter("w2", [2 * C1, 6 * 2 * C1], F32, isOutput=False)
    w3_d = nc.declare_dram_parameter("w3", [2 * C1, 6 * C2], F32, isOutput=False)
    w4_d = nc.declare_dram_parameter("w4", [C2, 9 * C2], F32, isOutput=False)
    cw_dt = {"bf16": BF16, "f32r": F32, "f32": F32}[EXPERT_DT]
    cw_d = nc.declare_dram_parameter("cw", [C2, 8 * 9 * 128], cw_dt, isOutput=False)
    scl_d = nc.declare_dram_parameter("scl", [128, 12], F32, isOutput=False)
    # scl columns: 0=s1,1=t1 (rows 0:64), 2=s2,3=t2 (rows 0:64),
    #              4=s3,5=t3, 6=s4,7=t4, 8..11 unused
    scle_d = nc.declare_dram_parameter("scle", [128, 8], F32, isOutput=False)
    biae_d = nc.declare_dram_parameter("biae", [128, 8], F32, isOutput=False)

    flat_o = nc.declare_dram_parameter("flat_out", [C2, n_img], F32, isOutput=True)
    zf_o = nc.declare_dram_parameter("zf_out", [128, E * 2 * n_img], F32, isOutput=True)

    with tile.TileContext(nc) as tc:
        with (
            tc.tile_pool(name="persist", bufs=1) as pp,
            tc.tile_pool(name="rows", bufs=3) as rowp,
            tc.tile_pool(name="tmp", bufs=3) as tmpp,
            tc.tile_pool(name="ebuf", bufs=2) as ebp,
            tc.tile_pool(name="psum", bufs=6, space="PSUM") as psp,
        ):
            # ---- persistent weight / scale tiles ----
            w1s = pp.tile([27, 2 * C1], F32, tag="w1s", name="w1s")
            w2s = pp.tile([2 * C1, 6 * 2 * C1], F32, tag="w2s", name="w2s")
            w3s = pp.tile([2 * C1, 6 * C2], F32, tag="w3s", name="w3s")
            w4s = pp.tile([C2, 9 * C2], F32, tag="w4s", name="w4s")
            cws = pp.tile([C2, 8 * 9 * 128], cw_dt, tag="cws", name="cws")
            scls = pp.tile([128, 12], F32, tag="scls", name="scls")
            scles = pp.tile([128, 8], F32, tag="scles", name="scles")
            biaes = pp.tile([128, 8], F32, tag="biaes", name="biaes")
            nc.sync.dma_start(w1s[:], w1_d[:])
            nc.sync.dma_start(w2s[:], w2_d[:])
            nc.sync.dma_start(w3s[:], w3_d[:])
            nc.sync.dma_start(w4s[:], w4_d[:])
            nc.sync.dma_start(cws[:], cw_d[:])
            nc.sync.dma_start(scls[:], scl_d[:])
            nc.sync.dma_start(scles[:], scle_d[:])
            nc.sync.dma_start(biaes[:], biae_d[:])

            s1 = scls[0:C1, 0:1]
            t1 = scls[0:C1, 1:2]
            s1b = scls[C1:128, 0:1]
            t1b = scls[C1:128, 1:2]
            s2 = scls[0:128, 2:3]
            t2 = scls[0:128, 3:4]
            s3 = scls[0:C2, 4:5]
            t3 = scls[0:C2, 5:6]
            s4 = scls[0:C2, 6:7]
            t4 = scls[0:C2, 7:8]

            # ---- persistent activations (double buffered, zero borders) ----
            t27 = [pp.tile([27, H, W], F32, tag=f"t27_{q}", name=f"t27_{q}") for q in range(2)]
            c1o = [pp.tile([2 * C1, H + 2, W + 2], F32, tag=f"c1o_{q}", name=f"c1o_{q}") for q in range(2)]
            c3i = [pp.tile([2 * C1, 34, 34], F32, tag=f"c3i_{q}", name=f"c3i_{q}") for q in range(2)]
            c4i = [pp.tile([C2, 34, 34], F32, tag=f"c4i_{q}", name=f"c4i_{q}") for q in range(2)]
            fpr = [pp.tile([C2, 2, 18, 18], F32, tag=f"fpr_{q}", name=f"fpr_{q}") for q in range(2)]
            if EXPERT_DT == "bf16":
                fprr = [
                    pp.tile([C2, 2, 18, 18], BF16, tag=f"fprr_{q}", name=f"fprr_{q}")
                    for q in range(2)
                ]
                cwsr = cws  # shipped as bf16 already
            elif EXPERT_DT == "f32r":
                fprr = [
                    pp.tile([C2, 2, 18, 18], F32R, tag=f"fprr_{q}", name=f"fprr_{q}")
                    for q in range(2)
                ]
                cwsr = pp.tile([C2, 8 * 9 * 128], F32R, tag="cwsr", name="cwsr")
                nc.vector.tensor_copy(cwsr[:], cws[:])
            else:
                fprr = fpr
                cwsr = cws
            for q in range(2):
                nc.gpsimd.memset(t27[q][:], 0.0)
                nc.gpsimd.memset(c1o[q][:], 0.0)
                nc.gpsimd.memset(c3i[q][:], 0.0)
                nc.gpsimd.memset(c4i[q][:], 0.0)
                nc.gpsimd.memset(fpr[q][:], 0.0)
                if EXPERT_DT != "f32":
                    nc.gpsimd.memset(fprr[q][:], 0.0)

            flatb = pp.tile([C2, n_img], F32, tag="flatb", name="flatb")
            zfb = pp.tile([128, E, 2, n_img], F32, tag="zfb", name="zfb")

            for i in range(n_img):
                q = i % 2
                tt = t27[q]
                co = c1o[q]
                p3 = c3i[q]
                p4 = c4i[q]
                fp = fpr[i // 2 % 2]

                # ---- im2col for conv1: 9 shifted copies of the 3-ch image ----
                for k, (dy, dx) in enumerate(OFFS):
                    ly, hy = max(0, 1 - dy), min(H, H + 1 - dy)
                    lx, hx = max(0, 1 - dx), min(W, W + 1 - dx)
                    nc.sync.dma_start(
                        tt[3 * k : 3 * k + 3, ly:hy, lx:hx],
                        x_d[i, :, ly + dy - 1 : hy + dy - 1, lx + dx - 1 : hx + dx - 1],
                    )

                # ---- conv1: K=27 im2col matmul, 8 N-tiles of 512.
                # M=128: output channels duplicated; copy2 (parts 64:128)
                # is written one padded row earlier so co[64+c, r, x] ==
                # co[c, r+1, x], letting conv2 contract offset pairs
                # (dy=0, dy=1) in a single K=128 matmul. ----
                for n in range(8):
                    ps = psp.tile([128, 512], F32, tag="mm", name="mm")
                    nc.tensor.matmul(
                        ps[:],
                        w1s[:],
                        tt[:, 8 * n : 8 * n + 8, :],
                        start=True,
                        stop=True,
                    )
                    nc.scalar.activation(
                        co[0:C1, 1 + 8 * n : 9 + 8 * n, 1 : 1 + W],
                        ps[0:C1, :].rearrange("p (a b) -> p a b", a=8),
                        mybir.ActivationFunctionType.Relu,
                        bias=t1,
                        scale=s1,
                    )
                    nc.scalar.activation(
                        co[C1 : 2 * C1, 8 * n : 8 + 8 * n, 1 : 1 + W],
                        ps[C1:128, :].rearrange("p (a b) -> p a b", a=8),
                        mybir.ActivationFunctionType.Relu,
                        bias=t1b,
                        scale=s1b,
                    )

                # ---- conv2 (64->64) + maxpool -> c3i.
                # Offsets (0,dx)+(1,dx) pair into one K=128 matmul via the
                # shifted channel copy in co; (2,dx) are K=64 singles.
                # Output channels are duplicated again (M=128); the pooled
                # copy2 is written one pooled row earlier into c3i. ----
                for n in range(8):
                    ps = psp.tile([128, 512], F32, tag="mm", name="mm")
                    for dx in range(3):
                        nc.tensor.matmul(
                            ps[:],
                            w2s[:, dx * 128 : (dx + 1) * 128],
                            co[:, 8 * n : 8 * n + 8, dx : dx + W],
                            start=(dx == 0),
                            stop=False,
                        )
                    for dx in range(3):
                        nc.tensor.matmul(
                            ps[:],
                            w2s[0:C1, (3 + dx) * 128 : (4 + dx) * 128],
                            co[0:C1, 8 * n + 2 : 8 * n + 10, dx : dx + W],
                            start=False,
                            stop=(dx == 2),
                        )
                    c2r = rowp.tile([128, 8, W], F32, tag="c2r", name="c2r")
                    nc.scalar.activation(
                        c2r[:],
                        ps[:].rearrange("p (a b) -> p a b", a=8),
                        mybir.ActivationFunctionType.Relu,
                        bias=t2,
                        scale=s2,
                    )
                    tm = tmpp.tile([128, 8, 32], F32, tag="tm1", name="tm1")
                    c2rs = c2r[:].rearrange("p a (b two) -> p a b two", two=2)
                    nc.vector.tensor_max(tm[:], c2rs[:, :, :, 0], c2rs[:, :, :, 1])
                    tms = tm[:].rearrange("p (a two) b -> p a two b", two=2)
                    nc.vector.tensor_max(
                        p3[0:C1, 1 + 4 * n : 5 + 4 * n, 1:33],
                        tms[0:C1, :, 0, :],
                        tms[0:C1, :, 1, :],
                    )
                    nc.vector.tensor_max(
                        p3[C1 : 2 * C1, 4 * n : 4 + 4 * n, 1:33],
                        tms[C1:128, :, 0, :],
                        tms[C1:128, :, 1, :],
                    )

                # ---- conv3 (64->128): offset pairs via shifted copy ----
                for n in range(2):
                    ps = psp.tile([128, 512], F32, tag="mm", name="mm")
                    for dx in range(3):
                        nc.tensor.matmul(
                            ps[:],
                            w3s[:, dx * C2 : (dx + 1) * C2],
                            p3[:, 16 * n : 16 * n + 16, dx : dx + 32],
                            start=(dx == 0),
                            stop=False,
                        )
                    for dx in range(3):
                        nc.tensor.matmul(
                            ps[:],
                            w3s[0:C1, (3 + dx) * C2 : (4 + dx) * C2],
                            p3[0:C1, 16 * n + 2 : 16 * n + 18, dx : dx + 32],
                            start=False,
                            stop=(dx == 2),
                        )
                    nc.scalar.activation(
                        p4[:, 1 + 16 * n : 17 + 16 * n, 1:33],
                        ps[:].rearrange("p (a b) -> p a b", a=16),
                        mybir.ActivationFunctionType.Relu,
                        bias=t3,
                        scale=s3,
                    )

                # ---- conv4 (128->128) + maxpool -> fpr ----
                for n in range(2):
                    ps = psp.tile([128, 512], F32, tag="mm", name="mm")
                    for k, (dy, dx) in enumerate(OFFS):
                        nc.tensor.matmul(
                            ps[:],
                            w4s[:, k * C2 : (k + 1) * C2],
                            p4[:, 16 * n + dy : 16 * n + dy + 16, dx : dx + 32],
                            start=(k == 0),
                            stop=(k == 8),
                        )
                    c4r = rowp.tile([C2, 16, 32], F32, tag="c4r", name="c4r")
                    nc.scalar.activation(
                        c4r[:],
                        ps[:].rearrange("p (a b) -> p a b", a=16),
                        mybir.ActivationFunctionType.Relu,
                        bias=t4,
                        scale=s4,
                    )
                    tm = tmpp.tile([C2, 16, 16], F32, tag="tm2", name="tm2")
                    c4rs = c4r[:].rearrange("p a (b two) -> p a b two", two=2)
                    nc.vector.tensor_max(tm[:], c4rs[:, :, :, 0], c4rs[:, :, :, 1])
                    tms = tm[:].rearrange("p (a two) b -> p a two b", two=2)
                    nc.vector.tensor_max(
                        fp[:, q, 1 + 8 * n : 9 + 8 * n, 1:17],
                        tms[:, :, 0, :],
                        tms[:, :, 1, :],
                    )

                if q == 1:
                    # ---- per-pair: flat sums + dense expert conv heads ----
                    j = i // 2
                    nc.vector.reduce_sum(
                        flatb[:, 2 * j : 2 * j + 2],
                        fp[:, :, 1:17, 1:17],
                        axis=mybir.AxisListType.XY,
                    )
                    if EXPERT_DT != "f32":
                        fpx = fprr[i // 2 % 2]
                        nc.vector.tensor_copy(
                            fpx[:, :, 1:17, 1:17], fp[:, :, 1:17, 1:17]
                        )
                    else:
                        fpx = fp
                    for e in range(E):
                        for m in range(2):
                            g = (e * 2 + m) * 9
                            ps = psp.tile([128, 512], F32, tag="mm", name="mm")
                            for k, (dy, dx) in enumerate(OFFS):
                                nc.tensor.matmul(
                                    ps[:],
                                    cwsr[:, (g + k) * 128 : (g + k + 1) * 128],
                                    fpx[:, :, dy : dy + 16, dx : dx + 16],
                                    start=(k == 0),
                                    stop=(k == 8),
                                )
                            eb = ebp.tile([128, 2, 256], F32, tag="eb", name="eb")
                            em = e * 2 + m
                            nc.scalar.activation(
                                eb[:],
                                ps[:].rearrange("p (a b) -> p a b", a=2),
                                mybir.ActivationFunctionType.Relu,
                                bias=biaes[:, em : em + 1],
                                scale=scles[:, em : em + 1],
                            )
                            nc.vector.reduce_sum(
                                zfb[:, e, m, 2 * j : 2 * j + 2],
                                eb[:],
                                axis=mybir.AxisListType.X,
                            )

            nc.sync.dma_start(flat_o[:], flatb[:])
            nc.sync.dma_start(zf_o[:], zfb[:].rearrange("p a b c -> p (a b c)"))

    split_multi_waits(nc)
    return nc


# ----------------------------------------------------------------------------
# Host-side parameter preparation
# ----------------------------------------------------------------------------
def host_prep(params):
    p = {k: np.asarray(v, dtype=np.float32) for k, v in params.items()}

    def fold(g, be, b):
        s = (g / np.sqrt(np.float32(1.0) + np.float32(EPS))).astype(np.float32)
        t = (b * s + be).astype(np.float32)
        return s, t

    s1, t1 = fold(p["g1"], p["be1"], p["b1"])
    s2, t2 = fold(p["g2"], p["be2"], p["b2"])
    s3, t3 = fold(p["g3"], p["be3"], p["b3"])
    s4, t4 = fold(p["g4"], p["be4"], p["b4"])

    scl = np.zeros((128, 12), np.float32)
    scl[:C1, 0] = s1
    scl[:C1, 1] = t1
    scl[C1:, 0] = s1  # duplicated-channel copy (conv1 M=128)
    scl[C1:, 1] = t1
    scl[:C1, 2] = s2
    scl[:C1, 3] = t2
    scl[C1:, 2] = s2  # duplicated-channel copy (conv2 M=128)
    scl[C1:, 3] = t2
    scl[:C2, 4] = s3
    scl[:C2, 5] = t3
    scl[:C2, 6] = s4
    scl[:C2, 7] = t4

    # expert conv scale/bias: columns e*2+m over the 128 channels of m-tile
    se = (p["cg"] / np.sqrt(np.float32(1.0) + np.float32(EPS))).astype(np.float32)
    te = (p["cb"] * se + p["cbe"]).astype(np.float32)
    scle = np.zeros((128, 8), np.float32)
    biae = np.zeros((128, 8), np.float32)
    for e in range(E):
        for m in range(2):
            scle[:, e * 2 + m] = se[e, m * 128 : (m + 1) * 128]
            biae[:, e * 2 + m] = te[e, m * 128 : (m + 1) * 128]

    # t27 partition layout is (k, c): row k*3+c; conv1 M duplicated to 128
    w1 = np.ascontiguousarray(p["w1"].transpose(2, 3, 1, 0).reshape(27, C1))
    w1 = np.tile(w1, (1, 2))
    # conv2/conv3 lhsT: 3 offset-pair blocks [(dy=0,dx)+(dy=1,dx), K=128]
    # followed by 3 single blocks [(dy=2,dx), K=64 in rows 0:64].
    def pair_layout(w, dup_m):
        O = w.shape[0]
        blocks = []
        for dx in range(3):
            blk = np.concatenate([w[:, :, 0, dx].T, w[:, :, 1, dx].T], axis=0)
            blocks.append(np.tile(blk, (1, 2)) if dup_m else blk)
        for dx in range(3):
            sng = np.zeros((2 * C1, 2 * O if dup_m else O), np.float32)
            s = w[:, :, 2, dx].T
            sng[:C1] = np.tile(s, (1, 2)) if dup_m else s
            blocks.append(sng)
        return np.ascontiguousarray(np.concatenate(blocks, axis=1))

    w2 = pair_layout(p["w2"], dup_m=True)   # [128, 768]
    w3 = pair_layout(p["w3"], dup_m=False)  # [128, 768]
    w4 = np.ascontiguousarray(p["w4"].transpose(1, 2, 3, 0).reshape(C2, 9 * C2))
    # cw [E, 256, 128, 3, 3] -> [i, (e, m, k, o)]
    cw = p["cw"].reshape(E, 2, 128, C2, 3, 3)  # [e, m, o, i, dy, dx]
    cw = cw.transpose(3, 0, 1, 4, 5, 2)  # [i, e, m, dy, dx, o]
    cw = np.ascontiguousarray(cw.reshape(C2, 8 * 9 * 128))
    if EXPERT_DT == "bf16":
        import ml_dtypes

        cw = cw.astype(ml_dtypes.bfloat16)

    return {
        "w1": w1,
        "w2": w2,
        "w3": w3,
        "w4": w4,
        "cw": cw,
        "scl": scl,
        "scle": scle,
        "biae": biae,
    }, p


# ----------------------------------------------------------------------------
# Host-side tail: scorer, waterfall routing, expert FC heads, combine
# ----------------------------------------------------------------------------
def waterfall(scores_noisy):
    Bn, En = scores_noisy.shape
    NEG = np.float32(-1e9)
    assigned = np.zeros(Bn, bool)
    assignment = np.zeros((Bn, En), bool)
    cap = np.zeros(En, np.int64)
    for it in range(15):
        full = cap >= CAP
        masked = np.where(full[None, :], NEG - np.float32(1.0), scores_noisy)
        masked = np.where(full[None, :], -np.inf, masked)
        best = np.argmax(masked, axis=1)
        quota = 2**it
        for e in range(En):
            want = (~assigned) & (best == e)
            space = min(CAP - cap[e], quota)
            rank = np.cumsum(want.astype(np.int64)) - 1
            sel = want & (rank < space)
            assignment[:, e] |= sel
            cap[e] += sel.sum()
            assigned |= sel
    least = int(np.argmin(cap))
    assignment[:, least] |= ~assigned
    return assignment


def host_tail(flat_sums, zf_sums, p):
    """flat_sums: [B, C2] pixel sums; zf_sums: [E, B, C3] pixel sums."""
    flat = (flat_sums / np.float32(256.0)).astype(np.float32)
    zf = (zf_sums / np.float32(256.0)).astype(np.float32)

    hs = np.einsum("bc,ehc->beh", flat, p["spw"], dtype=np.float32) + p["spb"][None]
    scores = (
        np.einsum("beh,eh->be", hs, p["shw"][:, 0, :], dtype=np.float32)
        + p["shb"][None, :, 0]
    ).astype(np.float32)
    scores = scores - scores.mean(axis=0, keepdims=True)
    combined = scores + np.log(np.float32(1e-9))
    scores_noisy = np.clip(combined / np.float32(T_ROUTE), -1e9, 1e9).astype(
        np.float32
    )

    assignment = waterfall(scores_noisy)

    out = np.zeros((B, NC_CLS), np.float32)
    for e in range(E):
        toks = np.nonzero(assignment[:, e])[0]
        if toks.size == 0:
            continue
        z = np.maximum(zf[e, toks] @ p["pw"][e].T + p["pb"][e], 0.0).astype(np.float32)
        out[toks] = (z @ p["clw"][e].T + p["clb"][e]).astype(np.float32)
    return out


# ----------------------------------------------------------------------------
# Entry point
# ----------------------------------------------------------------------------
_CACHE = {}


def _get_nc():
    if "nc" not in _CACHE:
        _CACHE["nc"] = build_nc(N_IMG_LAUNCH)
    return _CACHE["nc"]


def run_device(x, prep):
    """Run the full batch through the device in several SPMD launches.

    Launch L covers images [L*8*nl, (L+1)*8*nl): core c takes the c-th
    nl-image slice of that window.  Returns (flat_sums [B,C2],
    zf_sums [E,B,C3], per-launch wall times).
    """
    nc = _get_nc()
    nl = N_IMG_LAUNCH
    x = np.ascontiguousarray(np.asarray(x, dtype=np.float32))
    flat_sums = np.zeros((B, C2), np.float32)
    zf_sums = np.zeros((E, B, C3), np.float32)
    n_launch = B // (N_CORES * nl)
    times = []
    for L in range(n_launch):
        base = L * N_CORES * nl
        in_maps = []
        for c in range(N_CORES):
            m = dict(prep)
            m["x"] = np.ascontiguousarray(x[base + c * nl : base + (c + 1) * nl])
            in_maps.append(m)
        t0 = time.perf_counter()
        results = run_bass_kernel_spmd(nc, in_maps, list(range(N_CORES))).results
        times.append(time.perf_counter() - t0)
        for c in range(N_CORES):
            lo = base + c * nl
            fo = results[c]["flat_out"]  # [C2, nl]
            zo = results[c]["zf_out"].reshape(128, E, 2, nl)
            flat_sums[lo : lo + nl] = fo.T
            for e in range(E):
                for m2 in range(2):
                    zf_sums[e, lo : lo + nl, m2 * 128 : (m2 + 1) * 128] = zo[
                        :, e, m2, :
                    ].T
    return flat_sums, zf_sums, times


def kernel(x, params):
    prep, p = host_prep(params)
    flat_sums, zf_sums, _ = run_device(x, prep)
    return host_tail(flat_sums, zf_sums, p)
